# revision 48
# baseline (speedup 1.0000x reference)
"""Trainium2 Bass kernel for 8-head MultiHeadAttention (B=4, S=2048, D=512).

Sharding: tensor-parallel over heads -- core c owns head c; host sums the 8
bf16 partial y's (fp64) and adds bo.  All matmul operands are bf16 (cost
model: 1 cycle/row at ANY moving size vs fp32r's >=256 limit; halves DMA).

Work is organized in 8 "units" (batch, q-half of 1024 tokens) x 16 slots
(k-chunks).  Per slot of unit u, in PE program order:
  logits(u,kc):  pl[128k,1024q] = kvt[0:64,kc].T @ qt-half (2x512 matmuls)
  exp on ACT:    et[(u,kc)] = exp(pl) -> [128,1024] bf16
  AV(u-1,qc):    po[128q,64] += et-chunk.T @ V[128k,64], every 2nd slot,
                 plus N=1 ones-matmuls accumulating sumexp into po[:,64]
                 (both halve the classic V'-stationary AV's moving rows)
  scale+trans:   po * recip(sumexp) -> stg pair (DVE), then a 53ns PE
                 transpose (identity matmul) -> ot[128d,128q] via PSUM
  y(u-2,chunk):  ot-chunk.T @ wo2 -> py[128tok,512] -> bf16 evac on DVE
  proj piece:    one KV/Q projection group of a later batch (schedule
                 spread over units 0-5 with per-k-chunk deadlines)

ACT does ONLY exp: 131072 columns = 132.9 us busy, the floor engine; the
sim cost is paced by it.  Keys to the schedule (found via TimelineSim
traces): NEVER let wait-heavy DMAs head-of-line-block the SP queue (the
XBAR V'-transposes are scheduled just-in-time; output transposes run on
the PE instead of the XBAR), keep input DMAs 4+ slots ahead, and keep the
last unit lean so the final exps are not delayed.  GPSIMD cannot touch
PSUM (HW rule), so it only runs the y DMAs; all PSUM evacs are DVE.
PSUM: lp 2x[128,1024] (4 banks) + pp 2x(po/tp) (2) + pa 2x[128,512] (2).

Measured: rel err 5.2e-3 on HW; cost model 162.0 us/core (vs 203.8
baseline, 1.26x): ACT 132.9 busy, PE ~129, DVE ~85, DMA ~77.  The first
slot is hand-interleaved (logits half 0 right after Q(0,0) so it does not
queue behind Q(0,1)'s DMA on the in-order PE) and 14 dummy warm-up
matmuls ramp the PE p-state during the initial DMA wait.  Remaining
non-overlapped edges: ~10 us startup chain (framework preamble + serial
wkv/x00/q00 transfers) and ~14 us post-exp tail (av+y of the last unit +
exit drain).  Tail y rows ship as per-chunk SP DMAs (short final drain
chain) instead of 512-token groups.
"""

import numpy as np

import concourse.bass as bass
import concourse.mybir as mybir
from concourse.tile import TileContext
from concourse.bass_utils import run_bass_kernel_spmd

# ---------------------------------------------------------------------------
# Workaround: this container's walrus rejects >1 sync wait on an InstDrain
# (TPB_CTRL). Split the TileContext exit-drain waits across single-wait NOPs.
_PATCHED = False


def _install_drain_patch():
    global _PATCHED
    if _PATCHED:
        return
    from concourse.vector_clock import ScopedClock, VectorClock

    def _split_drain_and_barrier(self, tick_clock, wait_clock):
        g = tick_clock.global_clock
        n = len(g)
        for i in range(n):
            t = g[i]
            if t > 0:
                vec = [0] * n
                vec[i] = t
                nop = self.nc.sync.nop(nofuse=True, hint=f"drain_wait_p{i}")
                wait_clock.add_sem_waits(
                    nop.ins, ScopedClock({None: VectorClock(vec)})
                )
        self.nc.sync.drain()
        self.nc.all_engine_barrier()
        assert self.sems is not None
        popped = self.nc._tile_sem_poison_stack.pop()
        assert popped is self._sem_poison
        self.nc.clear_and_free_semaphores(list(self.sems.allocated().values()))
        self.nc.all_engine_barrier()

    TileContext._drain_and_barrier = _split_drain_and_barrier
    _PATCHED = True


def _split_multi_waits(nc):
    """This walrus accepts at most ONE sync wait per instruction. Hoist extra
    waits onto same-engine NOPs inserted immediately before the instruction
    (same-engine program order preserves semantics)."""
    n_split = 0
    for blk in nc.m.functions[0].blocks:
        il = blk.instructions
        i = 0
        while i < len(il):
            inst = il[i]
            try:
                si = inst.sync_info
            except AttributeError:
                si = None
            if si is not None and si.on_wait is not None and len(si.on_wait) > 1:
                waits = list(si.on_wait)
                for j, w in enumerate(waits[:-1]):
                    nop = mybir.InstNoOp(
                        name=f"{inst.name}_hw{j}",
                        sync_info=mybir.SyncInfo(on_wait=[w], on_update=[]),
                        bass_nofuse=True,
                        engine=inst.engine,
                    )
                    il.insert(i, nop)
                    i += 1
                inst.sync_info = mybir.SyncInfo(
                    on_wait=[waits[-1]], on_update=list(si.on_update)
                )
                n_split += 1
            i += 1
    return n_split


# ---------------------------------------------------------------------------
B, S, D, H = 4, 2048, 512, 8
Dh = D // H  # 64
T = B * S  # 8192
NCORES = 8

F32 = mybir.dt.float32
BF16 = mybir.dt.bfloat16
NP_BF16 = mybir.dt.np(BF16)

TT = 512  # projection token subtile
NTT = S // TT  # 4 subtiles per batch
NKC = S // 128  # 16 k-chunks per batch
NU = 2 * B  # 8 units (batch, q-half)
QW = 1024  # q-half width
EXP_FN = mybir.ActivationFunctionType.Exp
ALU = mybir.AluOpType

# av(u-1, qc) placement within unit u: slot kc -> qc
AV_SLOT = {6: 0, 7: 1, 8: 2, 9: 3, 10: 4, 11: 5, 12: 6, 13: 7}

# projection-piece schedule: (u, kc) -> list of ("dma"|"kv"|"q", b, tt).
# Leveled so each unit's PE time slightly exceeds ACT's 16.6us; honors
# deadlines (kvt[b] tt by its first logits slot, proj(b) before unit 2b)
# and >=2-slot DMA prefetch distance.
PROJ_SCHED = {
    (0, 0): [("kv", 0, 1)],
    (0, 1): [("dma", 0, 3)],
    (0, 2): [("kv", 0, 2)],
    (0, 5): [("q", 0, 2)],
    (0, 12): [("vt", 0, 0), ("dma", 1, 2)],
    (0, 6): [("kv", 0, 3), ("dma", 1, 0)],
    (0, 9): [("q", 0, 3), ("dma", 1, 1)],
    (1, 0): [("kv", 1, 0)],

    (1, 5): [("kv", 1, 1), ("dma", 2, 0)],
    (1, 7): [("q", 1, 1), ("dma", 2, 1)],
    (1, 2): [("q", 1, 0), ("dma", 1, 3)],
    (1, 11): [("kv", 1, 2)],
    (1, 13): [("q", 1, 2), ("dma", 2, 2)],

    (1, 15): [("kv", 1, 3)],
    (2, 0): [("q", 1, 3), ("dma", 2, 3)],
    (2, 14): [("vt", 1, 0)],

    (2, 4): [("kv", 2, 0)],
    (2, 6): [("q", 2, 0)],

    (2, 10): [("kv", 2, 1)],
    (2, 12): [("q", 2, 1)],
    (2, 14): [("dma", 2, 3)],
    (3, 0): [("kv", 2, 2)],
    (3, 2): [("q", 2, 2)],
    (3, 4): [("dma", 3, 0)],
    (3, 6): [("kv", 2, 3)],
    (3, 8): [("q", 2, 3)],
    (3, 10): [("dma", 3, 1)],
    (3, 13): [("dma", 3, 2)],
    (3, 15): [("vt", 2, 0)],
    (4, 0): [("kv", 3, 0)],
    (4, 2): [("q", 3, 0)],
    (4, 4): [("dma", 3, 3)],
    (4, 6): [("kv", 3, 1)],
    (4, 8): [("q", 3, 1)],
    (5, 0): [("kv", 3, 2)],
    (5, 3): [("q", 3, 2)],
    (5, 6): [("kv", 3, 3)],
    (5, 9): [("q", 3, 3)],
    (5, 14): [("vt", 3, 0)],
}

# V hi/lo in the AV accumulation for av(u') emitted in units 6,7 -- the
# late units have no projection work to fill PE.  Unit 0 (no AV/y either)
# fills with a logits hi/lo pass instead (zero extra DMA traffic).
VHILO_UNITS = {5}  # u' values
VHILO_BATCHES = [2, 3]


def _build() -> bass.Bass:
    nc = bass.Bass(name="mha")
    xT = nc.dram_tensor("xT", [4, 128, T], BF16, kind="ExternalInput")
    qT = nc.dram_tensor("qT", [4, 128, T], BF16, kind="ExternalInput")
    wkv = nc.dram_tensor("wkv", [4, 128, 128], BF16, kind="ExternalInput")
    bkv = nc.dram_tensor("bkv", [128, 1], F32, kind="ExternalInput")
    wq = nc.dram_tensor("wq", [4, 128, Dh], BF16, kind="ExternalInput")
    bq = nc.dram_tensor("bq", [Dh, 1], F32, kind="ExternalInput")
    wo2 = nc.dram_tensor("wo2", [128, D], BF16, kind="ExternalInput")
    ones = nc.dram_tensor("ones", [128, 1], BF16, kind="ExternalInput")
    iden = nc.dram_tensor("iden", [128, 128], BF16, kind="ExternalInput")
    y = nc.dram_tensor("y", [T, D], BF16, kind="ExternalOutput")

    with TileContext(nc) as tc:
        with (
            tc.tile_pool(name="const", bufs=1) as cpool,
            tc.tile_pool(name="kv", bufs=4) as kvpool,
            tc.tile_pool(name="xin", bufs=4) as xpool,
            tc.tile_pool(name="qin", bufs=5) as qpool,
            tc.tile_pool(name="et", bufs=33) as epool,
            tc.tile_pool(name="vp", bufs=4) as vpool,
            tc.tile_pool(name="stg", bufs=8) as spool,
            tc.tile_pool(name="ot", bufs=8) as opool,
            tc.tile_pool(name="rec", bufs=4) as rpool,
            tc.tile_pool(name="yout", bufs=4) as ypool,
            tc.tile_pool(name="lp", bufs=2, space="PSUM") as lp,
            tc.tile_pool(name="pp", bufs=2, space="PSUM") as pp,
            tc.tile_pool(name="pa", bufs=2, space="PSUM") as pa,
        ):
            # ---- constants ----
            wkv_sb = cpool.tile([128, 4, 128], BF16)
            wq_sb = cpool.tile([128, 4, Dh], BF16)
            wo_sb = cpool.tile([128, D], BF16)
            bkv_sb = cpool.tile([128, 1], F32)
            bq_sb = cpool.tile([Dh, 1], F32)
            ones_sb = cpool.tile([128, 1], BF16)
            iden_sb = cpool.tile([128, 128], BF16)
            nc.sync.dma_start(bkv_sb[:], bkv[:])
            nc.sync.dma_start(bq_sb[:], bq[:])
            nc.sync.dma_start(wkv_sb[:], wkv[:].rearrange("c p j -> p c j"))
            nc.gpsimd.dma_start(wo_sb[:], wo2[:])
            nc.gpsimd.dma_start(iden_sb[:], iden[:])
            nc.gpsimd.dma_start(ones_sb[:], ones[:])

            # PE p-state warm-up: dummy matmuls during the initial DMA wait
            # so the first real matmuls run at full clock (2.4GHz needs 3us
            # of continuous PE busy; cold start is 0.65/1.2GHz).
            dum_sb = cpool.tile([1, 512], BF16)
            nc.vector.memset(dum_sb[:], 0.0)
            dum_ps = pa.tile([1, 512], F32, tag="pa", name="dum_ps")
            for i in range(14):
                nc.tensor.matmul(
                    dum_ps[:], dum_sb[:, 0:1], dum_sb[:], start=True, stop=True
                )

            kvt = {}  # b -> [128, S] bf16 (rows 0:64 K^T, 64:128 V^T hi)
            qt_lo = {}  # b -> [64, S] bf16 (Q^T residual, batch 0 only)
            kvt_lo = {}  # b -> [64, S] bf16 (V^T residual)
            qt = {}  # b -> [64, S] bf16
            vp = {}  # b -> [128, 16, 64] bf16
            vp_lo = {}  # b -> same, residual
            et = {}  # (u, kc) -> [128, QW] bf16
            stg = {}  # (u, pair) -> [128, 256] bf16 (hi 0:128, lo 128:256)
            ot = {}  # (u, pair) -> [128, 2, 128] bf16
            rec = {}  # u -> [128, 8] f32
            ysb = {}  # (u, grp) -> [128, 4, 512] bf16
            xstage = {}
            qstage = {}

            def emit_in_dma(b, tt, q_first=False):
                t0 = b * S + tt * TT
                xt_t = xpool.tile([128, 4, TT], BF16, tag="xt", name=f"xt_{b}_{tt}")
                qt_t = qpool.tile([128, 4, TT], BF16, tag="qs", name=f"qs_{b}_{tt}")
                order = [
                    (qt_t, qT) if q_first else (xt_t, xT),
                    (xt_t, xT) if q_first else (qt_t, qT),
                ]
                for sb, dram in order:
                    nc.sync.dma_start(
                        sb[:], dram[:, :, t0 : t0 + TT].rearrange("c p j -> p c j")
                    )
                xstage[(b, tt)] = xt_t
                qstage[(b, tt)] = qt_t

            def emit_proj_kv(b, tt):
                if b not in kvt:
                    kvt[b] = kvpool.tile(
                        [128, S], BF16, tag="kvt", name=f"kvt_{b}", bufs=4
                    )
                    if b in VHILO_BATCHES:
                        kvt_lo[b] = kvpool.tile(
                            [128, S], BF16, tag="kvl", name=f"kvl_{b}", bufs=3
                        )
                kvp = pa.tile([128, TT], F32, tag="pa", name=f"kvp_{b}_{tt}")
                xt_t = xstage.pop((b, tt))
                for c in range(4):
                    nc.tensor.matmul(
                        kvp[:],
                        wkv_sb[:, c, :],
                        xt_t[:, c, :],
                        start=(c == 0),
                        stop=(c == 3),
                    )
                cols = slice(tt * TT, (tt + 1) * TT)
                nc.vector.tensor_scalar_add(kvt[b][:, cols], kvp[:], bkv_sb[:, 0:1])
                if b in VHILO_BATCHES:
                    # V residual: (kvp + bkv) - bf16(kvp + bkv), V rows only
                    nc.vector.scalar_tensor_tensor(
                        kvt_lo[b][64:128, cols],
                        kvp[64:128, :],
                        bkv_sb[64:128, 0:1],
                        kvt[b][64:128, cols],
                        ALU.add,
                        ALU.subtract,
                    )

            def emit_proj_q(b, tt):
                if b not in qt:
                    qt[b] = kvpool.tile(
                        [Dh, S], BF16, tag="qt", name=f"qt_{b}", bufs=4
                    )
                    if b == 0:
                        qt_lo[b] = kvpool.tile(
                            [Dh, S], BF16, tag="qtl", name=f"qtl_{b}", bufs=1
                        )
                qp = pa.tile([Dh, TT], F32, tag="pa", name=f"qp_{b}_{tt}")
                qt_t = qstage.pop((b, tt))
                for c in range(4):
                    nc.tensor.matmul(
                        qp[:],
                        wq_sb[:, c, :],
                        qt_t[:, c, :],
                        start=(c == 0),
                        stop=(c == 3),
                    )
                cols = slice(tt * TT, (tt + 1) * TT)
                nc.vector.tensor_scalar_add(qt[b][:, cols], qp[:], bq_sb[:, 0:1])
                if b == 0:
                    nc.vector.scalar_tensor_tensor(
                        qt_lo[b][:, cols],
                        qp[:],
                        bq_sb[:, 0:1],
                        qt[b][:, cols],
                        ALU.add,
                        ALU.subtract,
                    )

            def emit_vtrans(b):
                vp[b] = vpool.tile(
                    [128, NKC, Dh], BF16, tag="vp", name=f"vp_{b}", bufs=4
                )
                nc.sync.dma_start_transpose(vp[b][:], kvt[b][64:128, :])
                if b in VHILO_BATCHES:
                    vp_lo[b] = vpool.tile(
                        [128, NKC, Dh], BF16, tag="vpl", name=f"vpl_{b}", bufs=3
                    )
                    nc.sync.dma_start_transpose(vp_lo[b][:], kvt_lo[b][64:128, :])

            def emit_proj_piece(spec):
                kind, b, tt = spec
                if kind == "dma":
                    emit_in_dma(b, tt)
                elif kind == "kv":
                    emit_proj_kv(b, tt)
                elif kind == "vt":
                    emit_vtrans(b)
                else:
                    emit_proj_q(b, tt)

            def emit_logits_exp(u, kc):
                b, h = u // 2, u % 2
                et_t = epool.tile([128, QW], BF16, tag="et", name=f"et_{u}_{kc}")
                et[(u, kc)] = et_t
                pl = lp.tile([128, QW], F32, tag="lp", name=f"pl_{u}_{kc}")
                hilo = False
                kslice = kvt[b][0:64, kc * 128 : (kc + 1) * 128]
                split_first = u == 0 and kc == 0
                for hf in range(2):  # ISA: moving <= 512 elements per matmul
                    cols = slice(h * QW + hf * 512, h * QW + (hf + 1) * 512)
                    out = pl[:, hf * 512 : (hf + 1) * 512]
                    nc.tensor.matmul(
                        out, kslice, qt[b][:, cols], start=True, stop=not hilo
                    )
                    if hilo:
                        nc.tensor.matmul(
                            out, kslice, qt_lo[b][:, cols], start=False, stop=True
                        )
                    if split_first:
                        nc.scalar.activation(
                            et_t[:, hf * 512 : (hf + 1) * 512], out, EXP_FN
                        )
                if not split_first:
                    nc.scalar.activation(et_t[:], pl[:], EXP_FN)

            def emit_av(u, qc):
                # out[q, d] for q-chunk qc of unit u, contracting all 16 kc
                b = u // 2
                vhilo = u in VHILO_UNITS
                if qc == 0:
                    rec[u] = rpool.tile([128, 8], F32, tag="rec", name=f"rec_{u}")
                po = pp.tile([128, Dh + 1], F32, tag="pp", name=f"po_{u}_{qc}")
                q0 = qc * 128
                for kc in range(NKC):
                    nc.tensor.matmul(
                        po[:, 0:Dh],
                        et[(u, kc)][:, q0 : q0 + 128],
                        vp[b][:, kc, :],
                        start=(kc == 0),
                        stop=(kc == NKC - 1 and not vhilo),
                    )
                    if vhilo:
                        nc.tensor.matmul(
                            po[:, 0:Dh],
                            et[(u, kc)][:, q0 : q0 + 128],
                            vp_lo[b][:, kc, :],
                            start=False,
                            stop=(kc == NKC - 1),
                        )
                for kc in range(NKC):
                    nc.tensor.matmul(
                        po[:, Dh : Dh + 1],
                        et[(u, kc)][:, q0 : q0 + 128],
                        ones_sb[:],
                        start=(kc == 0),
                        stop=(kc == NKC - 1),
                    )
                nc.vector.reciprocal(rec[u][:, qc : qc + 1], po[:, Dh : Dh + 1])
                grp = qc // 2
                j = qc % 2
                if j == 0:
                    stg[(u, grp)] = spool.tile(
                        [128, 128], BF16, tag="stg", name=f"stg_{u}_{grp}"
                    )
                st = stg[(u, grp)]
                nc.vector.tensor_scalar_mul(
                    st[:, j * Dh : (j + 1) * Dh],
                    po[:, 0:Dh],
                    rec[u][:, qc : qc + 1],
                )
                if j == 1:
                    ot_t = opool.tile(
                        [128, 128], BF16, tag="ot", name=f"ot_{u}_{grp}"
                    )
                    ot[(u, grp)] = ot_t
                    tp = pp.tile([128, 128], BF16, tag="pp", name=f"tp_{u}_{grp}")
                    nc.tensor.transpose(tp[:], st[:], iden_sb[:])
                    nc.vector.tensor_copy(ot_t[:], tp[:])

            def emit_y(u, qc, tail=False):
                # output projection for 128-token chunk qc of unit u (hi+lo)
                b, h = u // 2, u % 2
                grp = qc // 4
                if qc % 4 == 0:
                    ysb[(u, grp)] = ypool.tile(
                        [128, 4, TT], BF16, tag="ysb", name=f"ysb_{u}_{grp}"
                    )
                ot_t = ot[(u, qc // 2)]
                j = qc % 2
                rows = slice(j * Dh, (j + 1) * Dh)
                g = u * 8 + qc
                if tail and g % 2 == 0:
                    # after the last exp the logits PSUM ring is free: use it
                    # to deepen the py pipeline for the drain
                    py = lp.tile([128, D], F32, tag="lp", name=f"py_{u}_{qc}")
                else:
                    py = pa.tile([128, D], F32, tag="pa", name=f"py_{u}_{qc}")
                nc.tensor.matmul(
                    py[:], ot_t[rows, :], wo_sb[rows, :], start=True, stop=True
                )
                nc.vector.tensor_copy(ysb[(u, grp)][:, qc % 4, :], py[:])
                if tail:
                    if u == NU - 2 and qc == 6:
                        # chunks 4,5 of this group were emitted in-loop and
                        # never group-DMA'd: ship 4,5,6 together
                        t0 = b * S + h * QW + 4 * 128
                        nc.sync.dma_start(
                            y[t0 : t0 + 384, :].rearrange(
                                "(j p) c -> p j c", p=128
                            ),
                            ysb[(u, grp)][:, 0:3, :],
                        )
                    else:
                        t0 = b * S + h * QW + qc * 128
                        nc.sync.dma_start(
                            y[t0 : t0 + 128, :], ysb[(u, grp)][:, qc % 4, :]
                        )
                elif qc % 4 == 3:
                    t0 = b * S + h * QW + grp * 512
                    nc.gpsimd.dma_start(
                        y[t0 : t0 + 512, :].rearrange("(j p) c -> p j c", p=128),
                        ysb[(u, grp)][:],
                    )

            # ---------------- pipeline ----------------
            emit_in_dma(0, 0)
            nc.sync.dma_start(wq_sb[:], wq[:].rearrange("c p j -> p c j"))
            emit_in_dma(0, 1, q_first=True)
            emit_in_dma(0, 2)
            emit_proj_kv(0, 0)
            emit_proj_q(0, 0)
            # interleaved first slot: logits/exp half 0 needs only Q(0,0);
            # Q(0,1) and half 1 follow (PE is in-order -- emitting Q(0,1)
            # first would stall the first exp on the q01 DMA)
            et00 = epool.tile([128, QW], BF16, tag="et", name="et_0_0")
            et[(0, 0)] = et00
            pl00 = lp.tile([128, QW], F32, tag="lp", name="pl_0_0")
            nc.tensor.matmul(
                pl00[:, 0:512], kvt[0][0:64, 0:128], qt[0][:, 0:512],
                start=True, stop=True,
            )
            nc.scalar.activation(et00[:, 0:512], pl00[:, 0:512], EXP_FN)
            emit_proj_q(0, 1)
            nc.tensor.matmul(
                pl00[:, 512:1024], kvt[0][0:64, 0:128], qt[0][:, 512:1024],
                start=True, stop=True,
            )
            nc.scalar.activation(et00[:, 512:1024], pl00[:, 512:1024], EXP_FN)

            for u in range(NU):
                for kc in range(NKC):
                    specs = PROJ_SCHED.get((u, kc), ())
                    for spec in specs:
                        if spec[0] == "dma":
                            emit_proj_piece(spec)
                    if (u, kc) != (0, 0):
                        emit_logits_exp(u, kc)
                    if u >= 1 and kc in AV_SLOT:
                        emit_av(u - 1, AV_SLOT[kc])
                    if u >= 2 and kc % 2 == 1:
                        emit_y(u - 2, (kc - 1) // 2)
                    if u == NU - 1 and 10 <= kc:
                        emit_y(u - 1, kc - 10)

                    for spec in specs:
                        if spec[0] != "dma":
                            emit_proj_piece(spec)

            # tail: attention of the last unit first (PE continuous), then
            # the remaining y chunks through a 4-deep py ring (pa + idle lp)
            for qc in range(8):
                emit_av(NU - 1, qc)
                if qc == 3:
                    emit_y(NU - 2, 6, tail=True)
                    emit_y(NU - 1, 0, tail=True)
                    emit_y(NU - 1, 1, tail=True)
                if qc == 5:
                    emit_y(NU - 2, 7, tail=True)
                    emit_y(NU - 1, 2, tail=True)
                    emit_y(NU - 1, 3, tail=True)
            for qc in range(4, 8):
                emit_y(NU - 1, qc, tail=True)

    _split_multi_waits(nc)
    return nc


_CACHE: dict = {}


def _prep_inputs(x, q, Wq, bq, Wk, bk, Wv, bv, Wo, bo):
    x = np.asarray(x, np.float32)
    q = np.asarray(q, np.float32)
    Wq, bq = np.asarray(Wq, np.float32), np.asarray(bq, np.float32)
    Wk, bk = np.asarray(Wk, np.float32), np.asarray(bk, np.float32)
    Wv, bv = np.asarray(Wv, np.float32), np.asarray(bv, np.float32)
    Wo = np.asarray(Wo, np.float32)

    scale = 1.0 / np.sqrt(np.float32(Dh))
    xT4 = np.ascontiguousarray(x.reshape(T, D).T.reshape(4, 128, T)).astype(NP_BF16)
    qT4 = np.ascontiguousarray(q.reshape(T, D).T.reshape(4, 128, T)).astype(NP_BF16)
    ones = np.ones((128, 1), dtype=NP_BF16)
    in_maps = []
    for h in range(NCORES):
        sl = slice(h * Dh, (h + 1) * Dh)
        wkv_h = np.concatenate([Wk[sl].T, Wv[sl].T], axis=1)  # [512, 128]
        bkv_h = np.concatenate([bk[sl], bv[sl]])[:, None]  # [128, 1]
        wq_h = (Wq[sl] * scale).T  # [512, 64]
        bq_h = (bq[sl] * scale)[:, None]
        wo_h = np.ascontiguousarray(Wo[:, sl].T)  # [64, 512]
        in_maps.append(
            {
                "xT": xT4,
                "qT": qT4,
                "wkv": np.ascontiguousarray(wkv_h.reshape(4, 128, 128)).astype(
                    NP_BF16
                ),
                "bkv": np.ascontiguousarray(bkv_h, dtype=np.float32),
                "wq": np.ascontiguousarray(wq_h.reshape(4, 128, Dh)).astype(NP_BF16),
                "bq": np.ascontiguousarray(bq_h, dtype=np.float32),
                "wo2": np.concatenate([wo_h, wo_h], axis=0).astype(NP_BF16),
                "ones": ones,
                "iden": np.eye(128, dtype=NP_BF16),
            }
        )
    return in_maps


def kernel(x, q, Wq, bq, Wk, bk, Wv, bv, Wo, bo):
    _install_drain_patch()
    if "nc" not in _CACHE:
        _CACHE["nc"] = _build()
    nc = _CACHE["nc"]
    in_maps = _prep_inputs(x, q, Wq, bq, Wk, bk, Wv, bv, Wo, bo)
    res = run_bass_kernel_spmd(nc, in_maps, core_ids=list(range(NCORES)))
    y = np.zeros((T, D), np.float64)
    for r in res.results:
        y += np.asarray(r["y"], dtype=np.float64)
    y = (y + np.asarray(bo, np.float32).astype(np.float64)).astype(np.float32)
    return y.reshape(B, S, D)


# revision 49
# speedup vs baseline: 1.0030x; 1.0030x over previous
"""Trainium2 Bass kernel for 8-head MultiHeadAttention (B=4, S=2048, D=512).

Sharding: tensor-parallel over heads -- core c owns head c; host sums the 8
bf16 partial y's (fp64) and adds bo.  All matmul operands are bf16 (cost
model: 1 cycle/row at ANY moving size vs fp32r's >=256 limit; halves DMA).

Work is organized in 8 "units" (batch, q-half of 1024 tokens) x 16 slots
(k-chunks).  Per slot of unit u, in PE program order:
  logits(u,kc):  pl[128k,1024q] = kvt[0:64,kc].T @ qt-half (2x512 matmuls)
  exp on ACT:    et[(u,kc)] = exp(pl) -> [128,1024] bf16
  AV(u-1,qc):    po[128q,64] += et-chunk.T @ V[128k,64], every 2nd slot,
                 plus N=1 ones-matmuls accumulating sumexp into po[:,64]
                 (both halve the classic V'-stationary AV's moving rows)
  scale+trans:   po * recip(sumexp) -> stg pair (DVE), then a 53ns PE
                 transpose (identity matmul) -> ot[128d,128q] via PSUM
  y(u-2,chunk):  ot-chunk.T @ wo2 -> py[128tok,512] -> bf16 evac on DVE
  proj piece:    one KV/Q projection group of a later batch (schedule
                 spread over units 0-5 with per-k-chunk deadlines)

ACT does ONLY exp: 131072 columns = 132.9 us busy, the floor engine; the
sim cost is paced by it.  Keys to the schedule (found via TimelineSim
traces): NEVER let wait-heavy DMAs head-of-line-block the SP queue (the
XBAR V'-transposes are scheduled just-in-time; output transposes run on
the PE instead of the XBAR), keep input DMAs 4+ slots ahead, and keep the
last unit lean so the final exps are not delayed.  GPSIMD cannot touch
PSUM (HW rule), so it only runs the y DMAs; all PSUM evacs are DVE.
PSUM: lp 2x[128,1024] (4 banks) + pp 2x(po/tp) (2) + pa 2x[128,512] (2).

Measured: rel err 5.2e-3 on HW; cost model 162.0 us/core (vs 203.8
baseline, 1.26x): ACT 132.9 busy, PE ~129, DVE ~85, DMA ~77.  The first
slot is hand-interleaved (logits half 0 right after Q(0,0) so it does not
queue behind Q(0,1)'s DMA on the in-order PE) and 14 dummy warm-up
matmuls ramp the PE p-state during the initial DMA wait.  Remaining
non-overlapped edges: ~10 us startup chain (framework preamble + serial
wkv/x00/q00 transfers) and ~14 us post-exp tail (av+y of the last unit +
exit drain).  Tail y rows ship as per-chunk SP DMAs (short final drain
chain) instead of 512-token groups.
"""

import numpy as np

import concourse.bass as bass
import concourse.mybir as mybir
from concourse.tile import TileContext
from concourse.bass_utils import run_bass_kernel_spmd

# ---------------------------------------------------------------------------
# Workaround: this container's walrus rejects >1 sync wait on an InstDrain
# (TPB_CTRL). Split the TileContext exit-drain waits across single-wait NOPs.
_PATCHED = False


def _install_drain_patch():
    global _PATCHED
    if _PATCHED:
        return
    from concourse.vector_clock import ScopedClock, VectorClock

    def _split_drain_and_barrier(self, tick_clock, wait_clock):
        g = tick_clock.global_clock
        n = len(g)
        for i in range(n):
            t = g[i]
            if t > 0:
                vec = [0] * n
                vec[i] = t
                nop = self.nc.sync.nop(nofuse=True, hint=f"drain_wait_p{i}")
                wait_clock.add_sem_waits(
                    nop.ins, ScopedClock({None: VectorClock(vec)})
                )
        self.nc.sync.drain()
        self.nc.all_engine_barrier()
        assert self.sems is not None
        popped = self.nc._tile_sem_poison_stack.pop()
        assert popped is self._sem_poison
        self.nc.clear_and_free_semaphores(list(self.sems.allocated().values()))
        self.nc.all_engine_barrier()

    TileContext._drain_and_barrier = _split_drain_and_barrier
    _PATCHED = True


def _split_multi_waits(nc):
    """This walrus accepts at most ONE sync wait per instruction. Hoist extra
    waits onto same-engine NOPs inserted immediately before the instruction
    (same-engine program order preserves semantics)."""
    n_split = 0
    for blk in nc.m.functions[0].blocks:
        il = blk.instructions
        i = 0
        while i < len(il):
            inst = il[i]
            try:
                si = inst.sync_info
            except AttributeError:
                si = None
            if si is not None and si.on_wait is not None and len(si.on_wait) > 1:
                waits = list(si.on_wait)
                for j, w in enumerate(waits[:-1]):
                    nop = mybir.InstNoOp(
                        name=f"{inst.name}_hw{j}",
                        sync_info=mybir.SyncInfo(on_wait=[w], on_update=[]),
                        bass_nofuse=True,
                        engine=inst.engine,
                    )
                    il.insert(i, nop)
                    i += 1
                inst.sync_info = mybir.SyncInfo(
                    on_wait=[waits[-1]], on_update=list(si.on_update)
                )
                n_split += 1
            i += 1
    return n_split


# ---------------------------------------------------------------------------
B, S, D, H = 4, 2048, 512, 8
Dh = D // H  # 64
T = B * S  # 8192
NCORES = 8

F32 = mybir.dt.float32
BF16 = mybir.dt.bfloat16
NP_BF16 = mybir.dt.np(BF16)

TT = 512  # projection token subtile
NTT = S // TT  # 4 subtiles per batch
NKC = S // 128  # 16 k-chunks per batch
NU = 2 * B  # 8 units (batch, q-half)
QW = 1024  # q-half width
EXP_FN = mybir.ActivationFunctionType.Exp
ALU = mybir.AluOpType

# av(u-1, qc) placement within unit u: slot kc -> qc
AV_SLOT = {6: 0, 7: 1, 8: 2, 9: 3, 10: 4, 11: 5, 12: 6, 13: 7}

# projection-piece schedule: (u, kc) -> list of ("dma"|"kv"|"q", b, tt).
# Leveled so each unit's PE time slightly exceeds ACT's 16.6us; honors
# deadlines (kvt[b] tt by its first logits slot, proj(b) before unit 2b)
# and >=2-slot DMA prefetch distance.
PROJ_SCHED = {
    (0, 0): [("kv", 0, 1)],
    (0, 1): [("dma", 0, 3)],
    (0, 2): [("kv", 0, 2)],
    (0, 5): [("q", 0, 2)],
    (0, 12): [("vt", 0, 0), ("dma", 1, 2)],
    (0, 6): [("kv", 0, 3), ("dma", 1, 0)],
    (0, 9): [("q", 0, 3), ("dma", 1, 1)],
    (1, 0): [("kv", 1, 0)],

    (1, 5): [("kv", 1, 1), ("dma", 2, 0)],
    (1, 7): [("q", 1, 1), ("dma", 2, 1)],
    (1, 2): [("q", 1, 0), ("dma", 1, 3)],
    (1, 11): [("kv", 1, 2)],
    (1, 13): [("q", 1, 2), ("dma", 2, 2)],

    (1, 15): [("kv", 1, 3)],
    (2, 0): [("q", 1, 3), ("dma", 2, 3)],
    (2, 14): [("vt", 1, 0)],

    (2, 4): [("kv", 2, 0)],
    (2, 6): [("q", 2, 0)],

    (2, 10): [("kv", 2, 1)],
    (2, 12): [("q", 2, 1)],
    (2, 14): [("dma", 2, 3)],
    (3, 0): [("kv", 2, 2)],
    (3, 2): [("q", 2, 2)],
    (3, 4): [("dma", 3, 0)],
    (3, 6): [("kv", 2, 3)],
    (3, 8): [("q", 2, 3)],
    (3, 10): [("dma", 3, 1)],
    (3, 13): [("dma", 3, 2)],
    (3, 15): [("vt", 2, 0)],
    (4, 0): [("kv", 3, 0)],
    (4, 2): [("q", 3, 0)],
    (4, 4): [("dma", 3, 3)],
    (4, 6): [("kv", 3, 1)],
    (4, 8): [("q", 3, 1)],
    (5, 0): [("kv", 3, 2)],
    (5, 3): [("q", 3, 2)],
    (5, 6): [("kv", 3, 3)],
    (5, 9): [("q", 3, 3)],
    (5, 14): [("vt", 3, 0)],
}

# V hi/lo in the AV accumulation for av(u') emitted in units 6,7 -- the
# late units have no projection work to fill PE.  Unit 0 (no AV/y either)
# fills with a logits hi/lo pass instead (zero extra DMA traffic).
VHILO_UNITS = {5}  # u' values
VHILO_BATCHES = [2, 3]


def _build() -> bass.Bass:
    nc = bass.Bass(name="mha")
    xT = nc.dram_tensor("xT", [4, 128, T], BF16, kind="ExternalInput")
    qT = nc.dram_tensor("qT", [4, 128, T], BF16, kind="ExternalInput")
    wkv = nc.dram_tensor("wkv", [4, 128, 128], BF16, kind="ExternalInput")
    bkv = nc.dram_tensor("bkv", [128, 1], F32, kind="ExternalInput")
    wq = nc.dram_tensor("wq", [4, 128, Dh], BF16, kind="ExternalInput")
    bq = nc.dram_tensor("bq", [Dh, 1], F32, kind="ExternalInput")
    wo2 = nc.dram_tensor("wo2", [128, D], BF16, kind="ExternalInput")
    ones = nc.dram_tensor("ones", [128, 1], BF16, kind="ExternalInput")
    iden = nc.dram_tensor("iden", [128, 128], BF16, kind="ExternalInput")
    y = nc.dram_tensor("y", [T, D], BF16, kind="ExternalOutput")

    with TileContext(nc) as tc:
        with (
            tc.tile_pool(name="const", bufs=1) as cpool,
            tc.tile_pool(name="kv", bufs=4) as kvpool,
            tc.tile_pool(name="xin", bufs=4) as xpool,
            tc.tile_pool(name="qin", bufs=5) as qpool,
            tc.tile_pool(name="et", bufs=33) as epool,
            tc.tile_pool(name="vp", bufs=4) as vpool,
            tc.tile_pool(name="stg", bufs=8) as spool,
            tc.tile_pool(name="ot", bufs=8) as opool,
            tc.tile_pool(name="rec", bufs=4) as rpool,
            tc.tile_pool(name="yout", bufs=4) as ypool,
            tc.tile_pool(name="lp", bufs=2, space="PSUM") as lp,
            tc.tile_pool(name="pp", bufs=2, space="PSUM") as pp,
            tc.tile_pool(name="pa", bufs=2, space="PSUM") as pa,
        ):
            # ---- constants ----
            wkv_sb = cpool.tile([128, 4, 128], BF16)
            wq_sb = cpool.tile([128, 4, Dh], BF16)
            wo_sb = cpool.tile([128, D], BF16)
            bkv_sb = cpool.tile([128, 1], F32)
            bq_sb = cpool.tile([Dh, 1], F32)
            ones_sb = cpool.tile([128, 1], BF16)
            iden_sb = cpool.tile([128, 128], BF16)
            nc.sync.dma_start(bkv_sb[:], bkv[:])
            nc.sync.dma_start(bq_sb[:], bq[:])
            nc.sync.dma_start(wkv_sb[:], wkv[:].rearrange("c p j -> p c j"))
            nc.gpsimd.dma_start(wo_sb[:], wo2[:])
            nc.gpsimd.dma_start(iden_sb[:], iden[:])
            nc.gpsimd.dma_start(ones_sb[:], ones[:])

            # PE p-state warm-up: dummy matmuls during the initial DMA wait
            # so the first real matmuls run at full clock (2.4GHz needs 3us
            # of continuous PE busy; cold start is 0.65/1.2GHz).
            dum_sb = cpool.tile([1, 512], BF16)
            nc.vector.memset(dum_sb[:], 0.0)
            dum_ps = pa.tile([1, 512], F32, tag="pa", name="dum_ps")
            for i in range(14):
                nc.tensor.matmul(
                    dum_ps[:], dum_sb[:, 0:1], dum_sb[:], start=True, stop=True
                )

            kvt = {}  # b -> [128, S] bf16 (rows 0:64 K^T, 64:128 V^T hi)
            qt_lo = {}  # b -> [64, S] bf16 (Q^T residual, batch 0 only)
            kvt_lo = {}  # b -> [64, S] bf16 (V^T residual)
            qt = {}  # b -> [64, S] bf16
            vp = {}  # b -> [128, 16, 64] bf16
            vp_lo = {}  # b -> same, residual
            et = {}  # (u, kc) -> [128, QW] bf16
            stg = {}  # (u, pair) -> [128, 256] bf16 (hi 0:128, lo 128:256)
            ot = {}  # (u, pair) -> [128, 2, 128] bf16
            rec = {}  # u -> [128, 8] f32
            ysb = {}  # (u, grp) -> [128, 4, 512] bf16
            xstage = {}
            qstage = {}

            def emit_in_dma(b, tt, q_first=False):
                t0 = b * S + tt * TT
                xt_t = xpool.tile([128, 4, TT], BF16, tag="xt", name=f"xt_{b}_{tt}")
                qt_t = qpool.tile([128, 4, TT], BF16, tag="qs", name=f"qs_{b}_{tt}")
                order = [
                    (qt_t, qT) if q_first else (xt_t, xT),
                    (xt_t, xT) if q_first else (qt_t, qT),
                ]
                for sb, dram in order:
                    nc.sync.dma_start(
                        sb[:], dram[:, :, t0 : t0 + TT].rearrange("c p j -> p c j")
                    )
                xstage[(b, tt)] = xt_t
                qstage[(b, tt)] = qt_t

            def emit_proj_kv(b, tt):
                if b not in kvt:
                    kvt[b] = kvpool.tile(
                        [128, S], BF16, tag="kvt", name=f"kvt_{b}", bufs=4
                    )
                    if b in VHILO_BATCHES:
                        kvt_lo[b] = kvpool.tile(
                            [128, S], BF16, tag="kvl", name=f"kvl_{b}", bufs=3
                        )
                kvp = pa.tile([128, TT], F32, tag="pa", name=f"kvp_{b}_{tt}")
                xt_t = xstage.pop((b, tt))
                for c in range(4):
                    nc.tensor.matmul(
                        kvp[:],
                        wkv_sb[:, c, :],
                        xt_t[:, c, :],
                        start=(c == 0),
                        stop=(c == 3),
                    )
                cols = slice(tt * TT, (tt + 1) * TT)
                nc.vector.tensor_scalar_add(kvt[b][:, cols], kvp[:], bkv_sb[:, 0:1])
                if b in VHILO_BATCHES:
                    # V residual: (kvp + bkv) - bf16(kvp + bkv), V rows only
                    nc.vector.scalar_tensor_tensor(
                        kvt_lo[b][64:128, cols],
                        kvp[64:128, :],
                        bkv_sb[64:128, 0:1],
                        kvt[b][64:128, cols],
                        ALU.add,
                        ALU.subtract,
                    )

            def emit_proj_q(b, tt):
                if b not in qt:
                    qt[b] = kvpool.tile(
                        [Dh, S], BF16, tag="qt", name=f"qt_{b}", bufs=4
                    )
                    if b == 0:
                        qt_lo[b] = kvpool.tile(
                            [Dh, S], BF16, tag="qtl", name=f"qtl_{b}", bufs=1
                        )
                qp = pa.tile([Dh, TT], F32, tag="pa", name=f"qp_{b}_{tt}")
                qt_t = qstage.pop((b, tt))
                for c in range(4):
                    nc.tensor.matmul(
                        qp[:],
                        wq_sb[:, c, :],
                        qt_t[:, c, :],
                        start=(c == 0),
                        stop=(c == 3),
                    )
                cols = slice(tt * TT, (tt + 1) * TT)
                nc.vector.tensor_scalar_add(qt[b][:, cols], qp[:], bq_sb[:, 0:1])
                if b == 0:
                    nc.vector.scalar_tensor_tensor(
                        qt_lo[b][:, cols],
                        qp[:],
                        bq_sb[:, 0:1],
                        qt[b][:, cols],
                        ALU.add,
                        ALU.subtract,
                    )

            def emit_vtrans(b):
                vp[b] = vpool.tile(
                    [128, NKC, Dh], BF16, tag="vp", name=f"vp_{b}", bufs=4
                )
                nc.sync.dma_start_transpose(vp[b][:], kvt[b][64:128, :])
                if b in VHILO_BATCHES:
                    vp_lo[b] = vpool.tile(
                        [128, NKC, Dh], BF16, tag="vpl", name=f"vpl_{b}", bufs=3
                    )
                    nc.sync.dma_start_transpose(vp_lo[b][:], kvt_lo[b][64:128, :])

            def emit_proj_piece(spec):
                kind, b, tt = spec
                if kind == "dma":
                    emit_in_dma(b, tt)
                elif kind == "kv":
                    emit_proj_kv(b, tt)
                elif kind == "vt":
                    emit_vtrans(b)
                else:
                    emit_proj_q(b, tt)

            def emit_logits_exp(u, kc):
                b, h = u // 2, u % 2
                et_t = epool.tile([128, QW], BF16, tag="et", name=f"et_{u}_{kc}")
                et[(u, kc)] = et_t
                pl = lp.tile([128, QW], F32, tag="lp", name=f"pl_{u}_{kc}")
                hilo = False
                kslice = kvt[b][0:64, kc * 128 : (kc + 1) * 128]
                split_first = u == 0 and kc == 0
                for hf in range(2):  # ISA: moving <= 512 elements per matmul
                    cols = slice(h * QW + hf * 512, h * QW + (hf + 1) * 512)
                    out = pl[:, hf * 512 : (hf + 1) * 512]
                    nc.tensor.matmul(
                        out, kslice, qt[b][:, cols], start=True, stop=not hilo
                    )
                    if hilo:
                        nc.tensor.matmul(
                            out, kslice, qt_lo[b][:, cols], start=False, stop=True
                        )
                    if split_first:
                        nc.scalar.activation(
                            et_t[:, hf * 512 : (hf + 1) * 512], out, EXP_FN
                        )
                if not split_first:
                    nc.scalar.activation(et_t[:], pl[:], EXP_FN)

            def emit_av(u, qc):
                # out[q, d] for q-chunk qc of unit u, contracting all 16 kc
                b = u // 2
                vhilo = u in VHILO_UNITS
                if qc == 0:
                    rec[u] = rpool.tile([128, 8], F32, tag="rec", name=f"rec_{u}")
                po = pp.tile([128, Dh + 1], F32, tag="pp", name=f"po_{u}_{qc}")
                q0 = qc * 128
                for kc in range(NKC):
                    nc.tensor.matmul(
                        po[:, 0:Dh],
                        et[(u, kc)][:, q0 : q0 + 128],
                        vp[b][:, kc, :],
                        start=(kc == 0),
                        stop=(kc == NKC - 1 and not vhilo),
                    )
                    if vhilo:
                        nc.tensor.matmul(
                            po[:, 0:Dh],
                            et[(u, kc)][:, q0 : q0 + 128],
                            vp_lo[b][:, kc, :],
                            start=False,
                            stop=(kc == NKC - 1),
                        )
                for kc in range(NKC):
                    nc.tensor.matmul(
                        po[:, Dh : Dh + 1],
                        et[(u, kc)][:, q0 : q0 + 128],
                        ones_sb[:],
                        start=(kc == 0),
                        stop=(kc == NKC - 1),
                    )
                nc.vector.reciprocal(rec[u][:, qc : qc + 1], po[:, Dh : Dh + 1])
                grp = qc // 2
                j = qc % 2
                if j == 0:
                    stg[(u, grp)] = spool.tile(
                        [128, 128], BF16, tag="stg", name=f"stg_{u}_{grp}"
                    )
                st = stg[(u, grp)]
                nc.vector.tensor_scalar_mul(
                    st[:, j * Dh : (j + 1) * Dh],
                    po[:, 0:Dh],
                    rec[u][:, qc : qc + 1],
                )
                if j == 1:
                    ot_t = opool.tile(
                        [128, 128], BF16, tag="ot", name=f"ot_{u}_{grp}"
                    )
                    ot[(u, grp)] = ot_t
                    tp = pp.tile([128, 128], BF16, tag="pp", name=f"tp_{u}_{grp}")
                    nc.tensor.transpose(tp[:], st[:], iden_sb[:])
                    nc.vector.tensor_copy(ot_t[:], tp[:])

            def emit_y(u, qc, tail=False):
                # output projection for 128-token chunk qc of unit u (hi+lo)
                b, h = u // 2, u % 2
                grp = qc // 4
                if qc % 4 == 0:
                    ysb[(u, grp)] = ypool.tile(
                        [128, 4, TT], BF16, tag="ysb", name=f"ysb_{u}_{grp}"
                    )
                ot_t = ot[(u, qc // 2)]
                j = qc % 2
                rows = slice(j * Dh, (j + 1) * Dh)
                g = u * 8 + qc
                if tail and g % 2 == 0:
                    # after the last exp the logits PSUM ring is free: use it
                    # to deepen the py pipeline for the drain
                    py = lp.tile([128, D], F32, tag="lp", name=f"py_{u}_{qc}")
                else:
                    py = pa.tile([128, D], F32, tag="pa", name=f"py_{u}_{qc}")
                nc.tensor.matmul(
                    py[:], ot_t[rows, :], wo_sb[rows, :], start=True, stop=True
                )
                nc.vector.tensor_copy(ysb[(u, grp)][:, qc % 4, :], py[:])
                if tail:
                    if u == NU - 2 and qc == 6:
                        # chunks 4,5 of this group were emitted in-loop and
                        # never group-DMA'd: ship 4,5,6 together
                        t0 = b * S + h * QW + 4 * 128
                        nc.sync.dma_start(
                            y[t0 : t0 + 384, :].rearrange(
                                "(j p) c -> p j c", p=128
                            ),
                            ysb[(u, grp)][:, 0:3, :],
                        )
                    else:
                        t0 = b * S + h * QW + qc * 128
                        nc.sync.dma_start(
                            y[t0 : t0 + 128, :], ysb[(u, grp)][:, qc % 4, :]
                        )
                elif qc % 4 == 3:
                    t0 = b * S + h * QW + grp * 512
                    nc.gpsimd.dma_start(
                        y[t0 : t0 + 512, :].rearrange("(j p) c -> p j c", p=128),
                        ysb[(u, grp)][:],
                    )

            # ---------------- pipeline ----------------
            emit_in_dma(0, 0)
            nc.sync.dma_start(wq_sb[:], wq[:].rearrange("c p j -> p c j"))
            emit_in_dma(0, 1, q_first=True)
            emit_in_dma(0, 2)
            emit_proj_kv(0, 0)
            emit_proj_q(0, 0)
            # interleaved first slot: logits/exp half 0 needs only Q(0,0);
            # Q(0,1) and half 1 follow (PE is in-order -- emitting Q(0,1)
            # first would stall the first exp on the q01 DMA)
            et00 = epool.tile([128, QW], BF16, tag="et", name="et_0_0")
            et[(0, 0)] = et00
            pl00 = lp.tile([128, QW], F32, tag="lp", name="pl_0_0")
            nc.tensor.matmul(
                pl00[:, 0:512], kvt[0][0:64, 0:128], qt[0][:, 0:512],
                start=True, stop=True,
            )
            nc.scalar.activation(et00[:, 0:512], pl00[:, 0:512], EXP_FN)
            emit_proj_q(0, 1)
            nc.tensor.matmul(
                pl00[:, 512:1024], kvt[0][0:64, 0:128], qt[0][:, 512:1024],
                start=True, stop=True,
            )
            nc.scalar.activation(et00[:, 512:1024], pl00[:, 512:1024], EXP_FN)

            for u in range(NU):
                for kc in range(NKC):
                    specs = PROJ_SCHED.get((u, kc), ())
                    for spec in specs:
                        if spec[0] == "dma":
                            emit_proj_piece(spec)
                    if (u, kc) != (0, 0):
                        emit_logits_exp(u, kc)
                    if u >= 1 and kc in AV_SLOT:
                        emit_av(u - 1, AV_SLOT[kc])
                    if u >= 2 and kc % 2 == 1:
                        emit_y(u - 2, (kc - 1) // 2)
                    if u == NU - 1 and 8 <= kc:
                        emit_y(u - 1, kc - 8)

                    for spec in specs:
                        if spec[0] != "dma":
                            emit_proj_piece(spec)

            # tail: attention of the last unit first (PE continuous), then
            # the remaining y chunks through a 4-deep py ring (pa + idle lp)
            for qc in range(8):
                emit_av(NU - 1, qc)
                if qc == 3:
                    emit_y(NU - 1, 0, tail=True)
                    emit_y(NU - 1, 1, tail=True)
                if qc == 5:
                    emit_y(NU - 1, 2, tail=True)
                    emit_y(NU - 1, 3, tail=True)
            for qc in range(4, 8):
                emit_y(NU - 1, qc, tail=True)

    _split_multi_waits(nc)
    return nc


_CACHE: dict = {}


def _prep_inputs(x, q, Wq, bq, Wk, bk, Wv, bv, Wo, bo):
    x = np.asarray(x, np.float32)
    q = np.asarray(q, np.float32)
    Wq, bq = np.asarray(Wq, np.float32), np.asarray(bq, np.float32)
    Wk, bk = np.asarray(Wk, np.float32), np.asarray(bk, np.float32)
    Wv, bv = np.asarray(Wv, np.float32), np.asarray(bv, np.float32)
    Wo = np.asarray(Wo, np.float32)

    scale = 1.0 / np.sqrt(np.float32(Dh))
    xT4 = np.ascontiguousarray(x.reshape(T, D).T.reshape(4, 128, T)).astype(NP_BF16)
    qT4 = np.ascontiguousarray(q.reshape(T, D).T.reshape(4, 128, T)).astype(NP_BF16)
    ones = np.ones((128, 1), dtype=NP_BF16)
    in_maps = []
    for h in range(NCORES):
        sl = slice(h * Dh, (h + 1) * Dh)
        wkv_h = np.concatenate([Wk[sl].T, Wv[sl].T], axis=1)  # [512, 128]
        bkv_h = np.concatenate([bk[sl], bv[sl]])[:, None]  # [128, 1]
        wq_h = (Wq[sl] * scale).T  # [512, 64]
        bq_h = (bq[sl] * scale)[:, None]
        wo_h = np.ascontiguousarray(Wo[:, sl].T)  # [64, 512]
        in_maps.append(
            {
                "xT": xT4,
                "qT": qT4,
                "wkv": np.ascontiguousarray(wkv_h.reshape(4, 128, 128)).astype(
                    NP_BF16
                ),
                "bkv": np.ascontiguousarray(bkv_h, dtype=np.float32),
                "wq": np.ascontiguousarray(wq_h.reshape(4, 128, Dh)).astype(NP_BF16),
                "bq": np.ascontiguousarray(bq_h, dtype=np.float32),
                "wo2": np.concatenate([wo_h, wo_h], axis=0).astype(NP_BF16),
                "ones": ones,
                "iden": np.eye(128, dtype=NP_BF16),
            }
        )
    return in_maps


def kernel(x, q, Wq, bq, Wk, bk, Wv, bv, Wo, bo):
    _install_drain_patch()
    if "nc" not in _CACHE:
        _CACHE["nc"] = _build()
    nc = _CACHE["nc"]
    in_maps = _prep_inputs(x, q, Wq, bq, Wk, bk, Wv, bv, Wo, bo)
    res = run_bass_kernel_spmd(nc, in_maps, core_ids=list(range(NCORES)))
    y = np.zeros((T, D), np.float64)
    for r in res.results:
        y += np.asarray(r["y"], dtype=np.float64)
    y = (y + np.asarray(bo, np.float32).astype(np.float64)).astype(np.float32)
    return y.reshape(B, S, D)
